# revision 8
# baseline (speedup 1.0000x reference)
"""Causal multi-head attention (B=2, S=2048, D=1024, H=16) on 8 TRN2 NeuronCores.

Sharding (data + tensor parallel, per the hint): core c handles batch b = c//4
and head-group g = c%4 (4 heads = 256 channels). Wq/Wk/Wv are split column-wise
(rows of the torch-layout weight) and Wo row-wise over those channels. Each core
computes a partial output [S, D]; the host sums the 4 group-partials per batch
and adds the bias.

Per-core pipeline (everything in transposed [channel, seq] space so no on-chip
transposes are needed; all matmuls use fp32r = full-rate FP22 multiplies):

  xT [D, S] (host-transposed)                          resident in SBUF
  qT/kT [o, S] = (wT-chunk).T @ xT                     o = 4 heads x 64
  v    [S, o]  = (xT-chunk).T @ wvT  (+ ones column)
  scoresT[kv, sq] = kT_h.T @ qT_h      per (128-kv-chunk, 512-sq-block),
                                       causally skipped; pairs share a
                                       2-bank PSUM tile
  p = exp(SCALE * scoresT)             one ACT op per pair, PSUM->SBUF
  causal mask on diagonal chunks       GPSIMD affine_select, fill 0
  ctxT[hd+1, sq] += v_chunk.T @ p      ones column accumulates the softmax
                                       denominator in row hd
  ctxT_norm = ctxT * (1/denom)         DVE; denom broadcast across partitions
                                       by an SBUF->SBUF DMA
  out[s, :] = sum_t ctxT-chunk.T @ woT-chunk           -> DRAM (partial)
"""

import sys

import numpy as np

sys.path.insert(0, "/opt/trn_rl_repo")

B, S, D, H = 2, 2048, 1024, 16
HD = 64
SCALE = 1.0 / float(np.sqrt(HD))
NCORES = 8
GROUPS = NCORES // B      # head-groups per batch (4)
HN = H // GROUPS          # heads per core (4)
O = HN * HD               # channels per core (256)

_CACHE = {}


def emit_mha(tc, out_d, xT_d, wqT_d, wkT_d, wvT_d, woT_d, *, seq, dmodel, hn, hd,
             scale):
    import concourse.mybir as mybir

    nc = tc.nc
    f32 = mybir.dt.float32
    f32r = mybir.dt.float32r
    EXP = mybir.ActivationFunctionType.Exp
    GE = mybir.AluOpType.is_ge

    o = hn * hd                # local qkv channels
    hpt = 128 // hd            # heads per qT/kT partition tile
    nqt = o // 128             # qT/kT partition tiles
    dc = dmodel // 128         # contraction chunks of the model dim
    sqb = min(512, seq)        # sq block = moving free dim of attention matmuls
    nj = seq // sqb            # sq blocks
    kcpb = sqb // 128          # kv chunks per sq block
    nn = (dmodel + 511) // 512 # out-proj free-dim chunks
    nw = dmodel // nn          # out-proj free chunk width

    assert o % 128 == 0 and dmodel % 128 == 0 and seq % sqb == 0
    assert kcpb % 2 == 0, "kv chunks per sq block must pair up"

    with (
        tc.tile_pool(name="persist", bufs=1) as pp,
        tc.tile_pool(name="work", bufs=3) as wp,
        tc.tile_pool(name="psum", bufs=2, space="PSUM") as psp,
        tc.tile_pool(name="dscr", bufs=2, space="DRAM") as dsp,
    ):
        # ---------------- DRAM loads ----------------
        def loadw(d_ap, nm):
            w = pp.tile([128, dc, o], f32r, name=nm, tag=nm)
            nc.sync.dma_start(out=w, in_=d_ap.rearrange("(t p) o -> p t o", p=128))
            return w

        wq = loadw(wqT_d, "wq")
        wk = loadw(wkT_d, "wk")
        wv = loadw(wvT_d, "wv")
        wo = []
        for t in range(nqt):
            wot = pp.tile([128, dmodel], f32r, name=f"wo{t}", tag=f"wo{t}")
            nc.sync.dma_start(out=wot, in_=woT_d[t * 128:(t + 1) * 128, :])
            wo.append(wot)

        xt = [pp.tile([128, seq], f32r, name=f"xt{t}", tag=f"xt{t}")
              for t in range(dc)]
        for j in range(nj):
            sl = slice(j * sqb, (j + 1) * sqb)
            for t in range(dc):
                nc.sync.dma_start(out=xt[t][:, sl],
                                  in_=xT_d[t * 128:(t + 1) * 128, sl])

        # ---------------- Q/K/V projections ----------------
        qt = [pp.tile([128, seq], f32r, name=f"qt{t}", tag=f"qt{t}")
              for t in range(nqt)]
        kt = [pp.tile([128, seq], f32r, name=f"kt{t}", tag=f"kt{t}")
              for t in range(nqt)]
        vt = [pp.tile([128, hn, hd + 1], f32r, name=f"vt{s}", tag=f"vt{s}")
              for s in range(seq // 128)]
        # memset can't target f32r (ISA check); stage the ones in f32 and
        # convert via tensor_copy.
        ones = pp.tile([128, hn], f32, name="ones", tag="ones")
        nc.vector.memset(ones, 1.0)

        for j in range(nj):
            sl = slice(j * sqb, (j + 1) * sqb)
            for t in range(nqt):
                for wsrc, dst, pn in ((wq, qt, "q"), (wk, kt, "k")):
                    ps = psp.tile([128, 2 * sqb], f32,
                                  name=f"ps_s_{pn}{t}_{j}", tag="ps_s")
                    for d in range(dc):
                        nc.tensor.matmul(
                            ps[:, 0:sqb],
                            lhsT=wsrc[:, d, t * 128:(t + 1) * 128],
                            rhs=xt[d][:, sl],
                            start=(d == 0), stop=(d == dc - 1))
                    nc.vector.tensor_copy(out=dst[t][:, sl], in_=ps[:, 0:sqb])
            for sc in range(j * kcpb, (j + 1) * kcpb):
                ps = psp.tile([128, o], f32, name=f"ps_c_v{sc}", tag="ps_c")
                for d in range(dc):
                    nc.tensor.matmul(
                        ps,
                        lhsT=xt[d][:, sc * 128:(sc + 1) * 128],
                        rhs=wv[:, d, :],
                        start=(d == 0), stop=(d == dc - 1))
                nc.vector.tensor_copy(
                    out=vt[sc][:, :, 0:hd],
                    in_=ps.rearrange("p (h e) -> p h e", h=hn))
                nc.vector.tensor_copy(
                    out=vt[sc][:, :, hd:hd + 1],
                    in_=ones.rearrange("p (h e) -> p h e", e=1))

        # ---------------- attention + output projection ----------------
        ctxt = [pp.tile([128, seq], f32r, name=f"ctxt{t}", tag=f"ctxt{t}")
                for t in range(nqt)]

        for j in range(nj):
            sl = slice(j * sqb, (j + 1) * sqb)
            for h in range(hn):
                tq, pq = divmod(h, hpt)
                base = pq * hd
                npairs = (j + 1) * kcpb // 2
                ps_ctx = psp.tile([128, sqb], f32, name=f"ps_c_x{j}_{h}",
                                  tag="ps_c")

                def scores_pair(p):
                    ps_s = psp.tile([128, 2 * sqb], f32,
                                    name=f"ps_s_a{j}_{h}_{p}", tag="ps_s")
                    pt = wp.tile([128, 2 * sqb], f32r, name=f"pt{j}_{h}_{p}",
                                 tag="pt")
                    for i in (0, 1):
                        c = 2 * p + i
                        nc.tensor.matmul(
                            ps_s[:, i * sqb:(i + 1) * sqb],
                            lhsT=kt[tq][base:base + hd,
                                        c * 128:(c + 1) * 128],
                            rhs=qt[tq][base:base + hd, sl],
                            start=True, stop=True)
                    nc.scalar.activation(out=pt, in_=ps_s, func=EXP, scale=scale)
                    for i in (0, 1):
                        c = 2 * p + i
                        if 128 * (c + 1) > j * sqb:  # diagonal chunk
                            nc.gpsimd.affine_select(
                                out=pt[:, i * sqb:(i + 1) * sqb],
                                in_=pt[:, i * sqb:(i + 1) * sqb],
                                compare_op=GE, fill=0.0,
                                base=j * sqb - c * 128,
                                channel_multiplier=-1,
                                pattern=[[1, sqb]])
                    return pt

                def pv_pair(p, pt, last):
                    for i in (0, 1):
                        c = 2 * p + i
                        nc.tensor.matmul(
                            ps_ctx[0:hd + 1, :],
                            lhsT=vt[c][:, h, :],
                            rhs=pt[:, i * sqb:(i + 1) * sqb],
                            start=(c == 0), stop=(last and i == 1))

                prev = None
                for p in range(npairs):
                    cur = scores_pair(p)
                    if prev is not None:
                        pv_pair(p - 1, prev, last=False)
                    prev = cur
                pv_pair(npairs - 1, prev, last=True)

                rc = wp.tile([1, sqb], f32, name=f"rc{j}_{h}", tag="rc", bufs=2)
                nc.vector.reciprocal(out=rc, in_=ps_ctx[hd:hd + 1, :])
                # SBUF APs need a nonzero partition step, so the partition
                # broadcast bounces through a DRAM scratch row.
                rd = dsp.tile([1, sqb], f32, name=f"rd{j}_{h}", tag="rd")
                nc.sync.dma_start(out=rd, in_=rc)
                rcb = wp.tile([hd, sqb], f32, name=f"rcb{j}_{h}", tag="rcb",
                              bufs=2)
                nc.sync.dma_start(out=rcb, in_=rd.to_broadcast((hd, sqb)))
                nc.vector.tensor_mul(ctxt[tq][base:base + hd, sl],
                                     ps_ctx[0:hd, :], rcb)

            for st in range(j * kcpb, (j + 1) * kcpb):
                for n in range(nn):
                    ps_o = psp.tile([128, nw], f32, name=f"ps_o{st}_{n}",
                                    tag="ps_o")
                    for t in range(nqt):
                        nc.tensor.matmul(
                            ps_o,
                            lhsT=ctxt[t][:, st * 128:(st + 1) * 128],
                            rhs=wo[t][:, n * nw:(n + 1) * nw],
                            start=(t == 0), stop=(t == nqt - 1))
                    ob = wp.tile([128, nw], f32, name=f"ob{st}_{n}", tag="ob",
                                 bufs=2)
                    nc.vector.tensor_copy(out=ob, in_=ps_o)
                    nc.sync.dma_start(
                        out=out_d[st * 128:(st + 1) * 128, n * nw:(n + 1) * nw],
                        in_=ob)


def build_nc(*, seq=S, dmodel=D, hn=HN, hd=HD, scale=SCALE, num_devices=NCORES):
    import concourse.mybir as mybir
    import concourse.tile as tile
    from concourse import bacc

    f32 = mybir.dt.float32
    o = hn * hd
    nc = bacc.Bacc("TRN2", target_bir_lowering=False, debug=False,
                   num_devices=num_devices)
    f32r = mybir.dt.float32r
    xT = nc.dram_tensor("xT", (dmodel, seq), f32r, kind="ExternalInput").ap()
    wqT = nc.dram_tensor("wqT", (dmodel, o), f32r, kind="ExternalInput").ap()
    wkT = nc.dram_tensor("wkT", (dmodel, o), f32r, kind="ExternalInput").ap()
    wvT = nc.dram_tensor("wvT", (dmodel, o), f32r, kind="ExternalInput").ap()
    woT = nc.dram_tensor("woT", (o, dmodel), f32r, kind="ExternalInput").ap()
    out = nc.dram_tensor("out", (seq, dmodel), f32, kind="ExternalOutput").ap()
    with tile.TileContext(nc) as tc:
        emit_mha(tc, out, xT, wqT, wkT, wvT, woT, seq=seq, dmodel=dmodel,
                 hn=hn, hd=hd, scale=scale)
    nc.compile()
    return nc


def make_in_maps(x, Wq, Wk, Wv, Wo):
    x = np.asarray(x, np.float32)
    Wq = np.asarray(Wq, np.float32)
    Wk = np.asarray(Wk, np.float32)
    Wv = np.asarray(Wv, np.float32)
    Wo = np.asarray(Wo, np.float32)
    in_maps = []
    for c in range(NCORES):
        b, g = divmod(c, GROUPS)
        ch = slice(g * O, (g + 1) * O)
        in_maps.append({
            "xT": np.ascontiguousarray(x[b].T),
            "wqT": np.ascontiguousarray(Wq[ch, :].T),
            "wkT": np.ascontiguousarray(Wk[ch, :].T),
            "wvT": np.ascontiguousarray(Wv[ch, :].T),
            "woT": np.ascontiguousarray(Wo[:, ch].T),
        })
    return in_maps


def combine_outputs(parts, bo):
    bo = np.asarray(bo, np.float64)
    out = np.empty((B, S, D), np.float32)
    for b in range(B):
        acc = np.zeros((S, D), np.float64)
        for g in range(GROUPS):
            acc += parts[b * GROUPS + g]
        out[b] = (acc + bo).astype(np.float32)
    return out


def run_on_hw(in_maps, **kwargs):
    from concourse import bass_utils
    if "nc" not in _CACHE:
        _CACHE["nc"] = build_nc()
    return bass_utils.run_bass_kernel_spmd(
        _CACHE["nc"], in_maps, core_ids=list(range(NCORES)), **kwargs)


def kernel(x, Wq, Wk, Wv, Wo, bo):
    res = run_on_hw(make_in_maps(x, Wq, Wk, Wv, Wo))
    parts = [res.results[c]["out"] for c in range(NCORES)]
    return combine_outputs(parts, bo)


# revision 9
# speedup vs baseline: 1.1569x; 1.1569x over previous
"""Causal multi-head attention (B=2, S=2048, D=1024, H=16) on 8 TRN2 NeuronCores.

Sharding (data + tensor parallel, per the hint): core c handles batch b = c//4
and head-group g = c%4 (4 heads = 256 channels). Wq/Wk/Wv are split column-wise
(rows of the torch-layout weight) and Wo row-wise over those channels. Each core
computes a partial output [S, D]; the host sums the 4 group-partials per batch
and adds the bias.

Per-core pipeline (everything in transposed [channel, seq] space so no on-chip
transposes are needed; all matmul operands are bf16 (fp32 PSUM accumulation)):

  xT [D, S] (host-transposed)                          resident in SBUF
  qT/kT [o, S] = (wT-chunk).T @ xT                     o = 4 heads x 64
  v    [S, o]  = (xT-chunk).T @ wvT  (+ ones column)
  scoresT[kv, sq] = kT_h.T @ qT_h      per (128-kv-chunk, 512-sq-block),
                                       causally skipped; pairs share a
                                       2-bank PSUM tile
  p = exp(SCALE * scoresT)             one ACT op per pair, PSUM->SBUF
  causal mask on diagonal chunks       GPSIMD affine_select, fill 0
  ctxT[hd+1, sq] += v_chunk.T @ p      ones column accumulates the softmax
                                       denominator in row hd
  ctxT_norm = ctxT * (1/denom)         DVE; denom broadcast across partitions
                                       by an SBUF->SBUF DMA
  out[s, :] = sum_t ctxT-chunk.T @ woT-chunk           -> DRAM (partial)
"""

import sys

import numpy as np

sys.path.insert(0, "/opt/trn_rl_repo")

B, S, D, H = 2, 2048, 1024, 16
HD = 64
SCALE = 1.0 / float(np.sqrt(HD))
NCORES = 8
GROUPS = NCORES // B      # head-groups per batch (4)
HN = H // GROUPS          # heads per core (4)
O = HN * HD               # channels per core (256)

_CACHE = {}


def emit_mha(tc, out_d, xT_d, wqT_d, wkT_d, wvT_d, woT_d, *, seq, dmodel, hn, hd,
             scale):
    import concourse.mybir as mybir

    nc = tc.nc
    f32 = mybir.dt.float32
    bf16 = mybir.dt.bfloat16
    EXP = mybir.ActivationFunctionType.Exp
    GE = mybir.AluOpType.is_ge

    o = hn * hd                # local qkv channels
    hpt = 128 // hd            # heads per qT/kT partition tile
    nqt = o // 128             # qT/kT partition tiles
    dc = dmodel // 128         # contraction chunks of the model dim
    sqb = min(512, seq)        # sq block = moving free dim of attention matmuls
    nj = seq // sqb            # sq blocks
    kcpb = sqb // 128          # kv chunks per sq block
    nn = (dmodel + 511) // 512 # out-proj free-dim chunks
    nw = dmodel // nn          # out-proj free chunk width

    assert o % 128 == 0 and dmodel % 128 == 0 and seq % sqb == 0
    assert kcpb % 2 == 0, "kv chunks per sq block must pair up"

    with (
        tc.tile_pool(name="persist", bufs=1) as pp,
        tc.tile_pool(name="work", bufs=3) as wp,
        tc.tile_pool(name="psum", bufs=2, space="PSUM") as psp,
        tc.tile_pool(name="dscr", bufs=2, space="DRAM") as dsp,
    ):
        # ---------------- DRAM loads ----------------
        def loadw(d_ap, nm):
            w = pp.tile([128, dc, o], bf16, name=nm, tag=nm)
            nc.sync.dma_start(out=w, in_=d_ap.rearrange("(t p) o -> p t o", p=128))
            return w

        wq = loadw(wqT_d, "wq")
        wk = loadw(wkT_d, "wk")
        wv = loadw(wvT_d, "wv")
        wo = []
        for t in range(nqt):
            wot = pp.tile([128, dmodel], bf16, name=f"wo{t}", tag=f"wo{t}")
            nc.sync.dma_start(out=wot, in_=woT_d[t * 128:(t + 1) * 128, :])
            wo.append(wot)

        xt = [pp.tile([128, seq], bf16, name=f"xt{t}", tag=f"xt{t}")
              for t in range(dc)]
        for j in range(nj):
            sl = slice(j * sqb, (j + 1) * sqb)
            for t in range(dc):
                nc.sync.dma_start(out=xt[t][:, sl],
                                  in_=xT_d[t * 128:(t + 1) * 128, sl])

        # ---------------- Q/K/V projections ----------------
        qt = [pp.tile([128, seq], bf16, name=f"qt{t}", tag=f"qt{t}")
              for t in range(nqt)]
        kt = [pp.tile([128, seq], bf16, name=f"kt{t}", tag=f"kt{t}")
              for t in range(nqt)]
        vt = [pp.tile([128, hn, hd + 1], bf16, name=f"vt{s}", tag=f"vt{s}")
              for s in range(seq // 128)]
        # memset can't target the matmul dtype directly on every ISA; stage
        # the ones in f32 and convert via tensor_copy.
        ones = pp.tile([128, hn], f32, name="ones", tag="ones")
        nc.vector.memset(ones, 1.0)

        for j in range(nj):
            sl = slice(j * sqb, (j + 1) * sqb)
            for t in range(nqt):
                for wsrc, dst, pn in ((wq, qt, "q"), (wk, kt, "k")):
                    ps = psp.tile([128, 2 * sqb], f32,
                                  name=f"ps_s_{pn}{t}_{j}", tag="ps_s")
                    for d in range(dc):
                        nc.tensor.matmul(
                            ps[:, 0:sqb],
                            lhsT=wsrc[:, d, t * 128:(t + 1) * 128],
                            rhs=xt[d][:, sl],
                            start=(d == 0), stop=(d == dc - 1))
                    nc.vector.tensor_copy(out=dst[t][:, sl], in_=ps[:, 0:sqb])
            for sc in range(j * kcpb, (j + 1) * kcpb):
                ps = psp.tile([128, o], f32, name=f"ps_c_v{sc}", tag="ps_c")
                for d in range(dc):
                    nc.tensor.matmul(
                        ps,
                        lhsT=xt[d][:, sc * 128:(sc + 1) * 128],
                        rhs=wv[:, d, :],
                        start=(d == 0), stop=(d == dc - 1))
                nc.vector.tensor_copy(
                    out=vt[sc][:, :, 0:hd],
                    in_=ps.rearrange("p (h e) -> p h e", h=hn))
                nc.vector.tensor_copy(
                    out=vt[sc][:, :, hd:hd + 1],
                    in_=ones.rearrange("p (h e) -> p h e", e=1))

        # ---------------- attention + output projection ----------------
        ctxt = [pp.tile([128, seq], bf16, name=f"ctxt{t}", tag=f"ctxt{t}")
                for t in range(nqt)]

        for j in range(nj):
            sl = slice(j * sqb, (j + 1) * sqb)
            for h in range(hn):
                tq, pq = divmod(h, hpt)
                base = pq * hd
                npairs = (j + 1) * kcpb // 2
                ps_ctx = psp.tile([128, sqb], f32, name=f"ps_c_x{j}_{h}",
                                  tag="ps_c")

                def scores_pair(p):
                    ps_s = psp.tile([128, 2 * sqb], f32,
                                    name=f"ps_s_a{j}_{h}_{p}", tag="ps_s")
                    pt = wp.tile([128, 2 * sqb], bf16, name=f"pt{j}_{h}_{p}",
                                 tag="pt")
                    for i in (0, 1):
                        c = 2 * p + i
                        nc.tensor.matmul(
                            ps_s[:, i * sqb:(i + 1) * sqb],
                            lhsT=kt[tq][base:base + hd,
                                        c * 128:(c + 1) * 128],
                            rhs=qt[tq][base:base + hd, sl],
                            start=True, stop=True)
                    nc.scalar.activation(out=pt, in_=ps_s, func=EXP, scale=scale)
                    for i in (0, 1):
                        c = 2 * p + i
                        if 128 * (c + 1) > j * sqb:  # diagonal chunk
                            nc.gpsimd.affine_select(
                                out=pt[:, i * sqb:(i + 1) * sqb],
                                in_=pt[:, i * sqb:(i + 1) * sqb],
                                compare_op=GE, fill=0.0,
                                base=j * sqb - c * 128,
                                channel_multiplier=-1,
                                pattern=[[1, sqb]])
                    return pt

                def pv_pair(p, pt, last):
                    for i in (0, 1):
                        c = 2 * p + i
                        nc.tensor.matmul(
                            ps_ctx[0:hd + 1, :],
                            lhsT=vt[c][:, h, :],
                            rhs=pt[:, i * sqb:(i + 1) * sqb],
                            start=(c == 0), stop=(last and i == 1))

                prev = None
                for p in range(npairs):
                    cur = scores_pair(p)
                    if prev is not None:
                        pv_pair(p - 1, prev, last=False)
                    prev = cur
                pv_pair(npairs - 1, prev, last=True)

                # Softmax denominators sit in one PSUM partition; a
                # single-lane DVE reciprocal over 512 elements measures
                # ~3.4us, so spread them over 128 partitions (via a DRAM
                # bounce, SBUF APs can't remap partitions), invert, and
                # bounce back for the partition broadcast.
                fw = sqb // 128
                dn = wp.tile([1, sqb], f32, name=f"dn{j}_{h}", tag="rc", bufs=2)
                nc.vector.tensor_copy(out=dn, in_=ps_ctx[hd:hd + 1, :])
                dd = dsp.tile([1, sqb], f32, name=f"dd{j}_{h}", tag="dd")
                nc.sync.dma_start(out=dd, in_=dn)
                rs = wp.tile([128, fw], f32, name=f"rs{j}_{h}", tag="rs", bufs=2)
                nc.sync.dma_start(
                    out=rs, in_=dd.rearrange("o (p f) -> (o p) f", p=128))
                nc.vector.reciprocal(out=rs, in_=rs)
                rd = dsp.tile([1, sqb], f32, name=f"rd{j}_{h}", tag="rd")
                nc.sync.dma_start(
                    out=rd.rearrange("o (p f) -> (o p) f", p=128), in_=rs)
                rcb = wp.tile([hd, sqb], f32, name=f"rcb{j}_{h}", tag="rcb",
                              bufs=2)
                nc.sync.dma_start(out=rcb, in_=rd.to_broadcast((hd, sqb)))
                nc.vector.tensor_mul(ctxt[tq][base:base + hd, sl],
                                     ps_ctx[0:hd, :], rcb)

            for st in range(j * kcpb, (j + 1) * kcpb):
                for n in range(nn):
                    ps_o = psp.tile([128, nw], f32, name=f"ps_o{st}_{n}",
                                    tag="ps_o")
                    for t in range(nqt):
                        nc.tensor.matmul(
                            ps_o,
                            lhsT=ctxt[t][:, st * 128:(st + 1) * 128],
                            rhs=wo[t][:, n * nw:(n + 1) * nw],
                            start=(t == 0), stop=(t == nqt - 1))
                    ob = wp.tile([128, nw], f32, name=f"ob{st}_{n}", tag="ob",
                                 bufs=2)
                    nc.vector.tensor_copy(out=ob, in_=ps_o)
                    nc.sync.dma_start(
                        out=out_d[st * 128:(st + 1) * 128, n * nw:(n + 1) * nw],
                        in_=ob)


def build_nc(*, seq=S, dmodel=D, hn=HN, hd=HD, scale=SCALE, num_devices=NCORES):
    import concourse.mybir as mybir
    import concourse.tile as tile
    from concourse import bacc

    f32 = mybir.dt.float32
    o = hn * hd
    nc = bacc.Bacc("TRN2", target_bir_lowering=False, debug=False,
                   num_devices=num_devices)
    bf16 = mybir.dt.bfloat16
    xT = nc.dram_tensor("xT", (dmodel, seq), bf16, kind="ExternalInput").ap()
    wqT = nc.dram_tensor("wqT", (dmodel, o), bf16, kind="ExternalInput").ap()
    wkT = nc.dram_tensor("wkT", (dmodel, o), bf16, kind="ExternalInput").ap()
    wvT = nc.dram_tensor("wvT", (dmodel, o), bf16, kind="ExternalInput").ap()
    woT = nc.dram_tensor("woT", (o, dmodel), bf16, kind="ExternalInput").ap()
    out = nc.dram_tensor("out", (seq, dmodel), f32, kind="ExternalOutput").ap()
    with tile.TileContext(nc) as tc:
        emit_mha(tc, out, xT, wqT, wkT, wvT, woT, seq=seq, dmodel=dmodel,
                 hn=hn, hd=hd, scale=scale)
    nc.compile()
    return nc


def make_in_maps(x, Wq, Wk, Wv, Wo):
    import ml_dtypes
    bf16 = ml_dtypes.bfloat16

    def cvt(a):
        return np.ascontiguousarray(np.asarray(a, np.float32)).astype(bf16)

    x = np.asarray(x, np.float32)
    Wq = np.asarray(Wq, np.float32)
    Wk = np.asarray(Wk, np.float32)
    Wv = np.asarray(Wv, np.float32)
    Wo = np.asarray(Wo, np.float32)
    in_maps = []
    for c in range(NCORES):
        b, g = divmod(c, GROUPS)
        ch = slice(g * O, (g + 1) * O)
        in_maps.append({
            "xT": cvt(x[b].T),
            "wqT": cvt(Wq[ch, :].T),
            "wkT": cvt(Wk[ch, :].T),
            "wvT": cvt(Wv[ch, :].T),
            "woT": cvt(Wo[:, ch].T),
        })
    return in_maps


def combine_outputs(parts, bo):
    bo = np.asarray(bo, np.float64)
    out = np.empty((B, S, D), np.float32)
    for b in range(B):
        acc = np.zeros((S, D), np.float64)
        for g in range(GROUPS):
            acc += parts[b * GROUPS + g]
        out[b] = (acc + bo).astype(np.float32)
    return out


def run_on_hw(in_maps, **kwargs):
    from concourse import bass_utils
    if "nc" not in _CACHE:
        _CACHE["nc"] = build_nc()
    return bass_utils.run_bass_kernel_spmd(
        _CACHE["nc"], in_maps, core_ids=list(range(NCORES)), **kwargs)


def kernel(x, Wq, Wk, Wv, Wo, bo):
    res = run_on_hw(make_in_maps(x, Wq, Wk, Wv, Wo))
    parts = [res.results[c]["out"] for c in range(NCORES)]
    return combine_outputs(parts, bo)
